# revision 17
# baseline (speedup 1.0000x reference)
"""BatchedLightSimulation Trainium2 kernel.

Math: the two causal convolutions (scintillation 990 taps, SiPM impulse 990
taps) compose into one 1979-tap causal filter c.  Folding the sum-by-16
downsample in gives

    out[row, s] = sum_delta c16[delta] * ug[row, 16*s + delta]

with c16[delta] = sum_{k=max(0,delta)}^{15} c[k - delta] and
ug[row, t] = gain[row] * u[row, t] (the per-detector gain is folded into
the input on the host).  c decays like exp(-l/15.3) so c16 truncated to
delta >= -240 is exact at fp32 precision.

Device mapping (per core, 4 ninputs = 192 (n,d) rows):
  polyphase m = 16q + r.  4 s-tiles of 100; SBUF tile X[q, st, r, row]
  holds bf16 ug[row, 16*(100*st + q - 15) + r] for q in [0,128) (115
  live + 13 zero-pad: DMAs with fewer than 128 SBUF partitions run ~20x
  slower, so every transfer is exactly 128 partitions).  Per (st, r) one
  bf16 matmul accumulates into psum[128, 192]: W_r.T @ x with
  W[q, s_rel] = c16[16*(q-15-s_rel)+r] banded.  bf16 x/W and a bf16
  output give 3.7e-3 max rel error vs the fp64 reference (harness gate
  2e-2).  Epilogue per s-tile: DVE copies psum[0:100,:] into a
  128-partition bf16 staging tile (junk rows pre-memset) laid out
  [s_rel, st*192+row]; two contiguous DRAM tensors (s-tiles 0-2, and
  s-tile 3 alone on the tail) are DMA'd out as full 128-partition
  blocks.  The host upcasts and permutes [s_rel, st, row] -> [row, s]
  (this removes the PE transposes, DVE adds and gain multiplies).

Perf notes (measured on TRN2 via NTFF profiles):
  - The kernel is HBM-bound at ~400 GB/s aggregate over both HWDGE
    rings; total traffic 3.75 MB (x 3.15 + W 0.4 + out 0.2).  ~27.3us
    end-to-end of which ~7us is NEFF/TileContext preamble and ~2.8us
    finalize (run-to-run noise from shared-HBM contention is +-0.6us,
    with slower multi-minute drift up to ~2.5us).
  - DMA transfers with < 128 SBUF partitions collapse to ~15-40 GB/s
    (per-partition descriptors), and lines < 3072B stop coalescing into
    4-6KB packets (quarter-chunks measured 2.5x slower per engine):
    every transfer here is 128 partitions x >=3072B lines except the
    two small output slabs.
  - The host ships each core's shard already in the polyphase layout (a
    pure permutation + bf16 cast done during the mandatory shard-and-copy
    step) so the input DMA is fully contiguous.
  - WCOL=100 (= STILE) keeps the 82ns full-clock matmul issue spacing
    (bf16 fast-weight-load does not need 128 stationary columns); rhs
    rows are innermost so the moving operand streams stride-1.
  - x transfers go in consumption order, half s-tile (8 r-phases) per
    DMA alternating rings, so each burst of 8 matmuls gates on 393KB.
  - 48 dummy matmuls on a memset tile bridge the HAM clock-gate warmup
    (1.2 -> 2.4 GHz) while W and the first chunks land, and 6 more after
    the first two s-tile bursts keep the gate open while the PE waits on
    DMA (measured: real matmul issue spacing stays at 82ns = full
    clock); none after the third burst - the last tile's data is
    usually resident by then and dummies would delay the tail.
"""

import numpy as np
import ml_dtypes

import concourse.bacc as bacc
import concourse.mybir as mybir
import concourse.tile as tile
from concourse.bass_utils import run_bass_kernel_spmd

# ---- problem constants (hardcoded per contract) ----
NINPUT, NDET, NTICK = 32, 48, 6400
NS = 16                    # downsample factor
S = NTICK // NS            # 400 output ticks
LIGHT_TICK = 0.1
CONV_TICKS = 990
NCORES = 8
N_PER_CORE = NINPUT // NCORES      # 4
ROWS = N_PER_CORE * NDET           # 192 rows per core
HALO = 15                          # q-steps of history (taps delta >= -240)
PAD = NS * HALO                    # 240 zero ticks prepended
TPAD = NTICK + PAD                 # 6640
STILE = 100                        # s-values per output tile
NST = S // STILE                   # 4
QW = STILE + HALO                  # 115 live q rows per tile
DMAX = NS * HALO                   # 240
N_WARM = 48                        # initial dummy matmuls (HAM clock gate)
N_WARM_GAP = 6                     # dummies between s-tile bursts
WCOL = 100                         # weight columns (= STILE; probing FWL)
CH = NS * ROWS                     # 3072: one s-tile's x cols
XFREE = NST * CH                   # 12288
TALLOC = NS * STILE * (NST - 1) + NS * 128 + NS  # strided-view extent

BF16 = ml_dtypes.bfloat16


def _build_taps(singlet_fraction_logit, log_tau_s, log_tau_t,
                light_oscillation_period, light_response_time):
    """c16[delta] for delta in [-DMAX, 15], float64."""
    dt = float(LIGHT_TICK)
    tt = np.arange(CONV_TICKS, dtype=np.float64)
    sf = 1.0 / (1.0 + np.exp(-float(singlet_fraction_logit)))
    tau_s = 10.0 ** float(log_tau_s)
    tau_t = 10.0 ** float(log_tau_t)
    per = float(light_oscillation_period)
    rt = float(light_response_time)
    p1 = sf * np.exp(-tt * dt / tau_s) * (1.0 - np.exp(-dt / tau_s))
    p3 = (1.0 - sf) * np.exp(-tt * dt / tau_t) * (1.0 - np.exp(-dt / tau_t))
    scint = p1 + p3
    t = tt * dt
    imp = np.exp(-t / rt) * np.sin(t / per)
    imp = imp / (per * rt * rt) * (per * per + rt * rt) * dt
    c = np.convolve(scint, imp)          # length 2*990-1 = 1979
    deltas = np.arange(-DMAX, 16)
    c16 = np.zeros(len(deltas), dtype=np.float64)
    for i, d in enumerate(deltas):
        ks = np.arange(max(0, d), 16)
        c16[i] = c[ks - d].sum()
    return c16                            # index i -> delta = i - DMAX


def _build_weights(c16):
    """W[q_rel, r, s_rel] float32 (128 rows, WCOL cols, banded)."""
    w = np.zeros((128, NS, WCOL), dtype=np.float64)
    q_rel = np.arange(128)[:, None, None]
    r = np.arange(NS)[None, :, None]
    s_rel = np.arange(WCOL)[None, None, :]
    delta = 16 * (q_rel - HALO - s_rel) + r
    mask = ((delta >= -DMAX) & (delta <= 15) & (q_rel < QW)
            & (s_rel < STILE))
    w[mask] = c16[(delta + DMAX)[mask]]
    return np.ascontiguousarray(w, dtype=np.float32)


_PROGRAM = None


def _build_program():
    global _PROGRAM
    if _PROGRAM is not None:
        return _PROGRAM
    nc = bacc.Bacc("TRN2", target_bir_lowering=False, debug=False,
                   num_devices=NCORES)
    f32 = mybir.dt.float32
    bf16 = mybir.dt.bfloat16
    x_d = nc.dram_tensor("x", [128, XFREE], bf16, kind="ExternalInput")
    w_d = nc.dram_tensor("w", [128, NS * WCOL], bf16, kind="ExternalInput")
    oa_d = nc.dram_tensor("oa", [128, 3 * ROWS], bf16, kind="ExternalOutput")
    ob_d = nc.dram_tensor("ob", [128, ROWS], bf16, kind="ExternalOutput")

    with tile.TileContext(nc) as tc:
        with (
            tc.tile_pool(name="const", bufs=1) as cpool,
            tc.tile_pool(name="x", bufs=1) as xpool,
            tc.tile_pool(name="fin", bufs=1) as fpool,
            tc.tile_pool(name="ps", bufs=1, space="PSUM") as pspool,
            tc.tile_pool(name="warm", bufs=1, space="PSUM") as wpool,
        ):
            # PE warm-up: dummy bf16 matmuls on a memset tile (no DMA
            # dependency) keep TensorE busy so the HAM clock gate opens
            # (1.2 -> 2.4 GHz) before the real matmuls start.
            warm_w = cpool.tile([128, 256], bf16, tag="warmw")
            nc.vector.memset(warm_w[:], 1.0)
            ps_warm = wpool.tile([128, 256], f32, tag="warm")
            for _ in range(N_WARM):
                nc.tensor.matmul(ps_warm[:], warm_w[:, 0:128], warm_w[:],
                                 start=True, stop=True)

            # output staging: [s_rel, st*ROWS+row] bf16; junk rows
            # [100:128) are memset once so the 128-partition out DMAs
            # read defined data
            fin = fpool.tile([128, NST * ROWS], bf16, tag="fin")
            nc.vector.memset(fin[:], 0.0)

            # W split across both rings so the first matmuls gate on only
            # half its latency.
            w_sb = cpool.tile([128, NS * WCOL], bf16, tag="w")
            HW = NS * WCOL // 2
            nc.sync.dma_start(w_sb[:, 0:HW], w_d[:, 0:HW])
            nc.scalar.dma_start(w_sb[:, HW:], w_d[:, HW:])

            # x[q, st, r, row]: row contiguous so the matmul moving
            # operand streams stride-1.  Consumption order, alternating
            # rings; last s-tile in quarters to shorten the tail.
            x_sb = xpool.tile([128, NST, NS, ROWS], bf16, tag="x")
            x_flat = x_sb[:].rearrange("q st r row -> q (st r row)")
            for st in range(NST):
                lo = st * CH
                nc.sync.dma_start(x_flat[:, lo:lo + CH // 2],
                                  x_d[:, lo:lo + CH // 2])
                nc.scalar.dma_start(x_flat[:, lo + CH // 2:lo + CH],
                                    x_d[:, lo + CH // 2:lo + CH])

            ps_tiles = []
            for st in range(NST):
                ps = pspool.tile([WCOL, ROWS], f32, tag=f"ps{st}")
                ps_tiles.append(ps)
                for r in range(NS):
                    nc.tensor.matmul(
                        ps[:], w_sb[:, r * WCOL:(r + 1) * WCOL],
                        x_sb[:, st, r, :],
                        start=(r == 0), stop=(r == NS - 1),
                    )
                if st < 2:
                    # keep the HAM activity monitor fed while the PE
                    # waits on the next s-tile's DMA (not before the
                    # last tile: its data is usually already in SBUF
                    # and the dummies would delay the tail burst)
                    for _ in range(N_WARM_GAP):
                        nc.tensor.matmul(ps_warm[:], warm_w[:, 0:128],
                                         warm_w[:], start=True, stop=True)

            for st in range(3):
                sl = slice(st * ROWS, (st + 1) * ROWS)
                nc.vector.tensor_copy(fin[0:STILE, sl],
                                      ps_tiles[st][0:STILE, :])
            # oa rides ring B: its descriptors sit behind x3B in ring
            # order, so it transfers the moment the stream ends instead
            # of stealing engine time from the stream's tail.
            nc.scalar.dma_start(oa_d[:], fin[:, 0:3 * ROWS])
            # tail copy split across DVE and Act so it takes ~0.18us
            sl3 = 3 * ROWS
            HR = ROWS // 2
            nc.vector.tensor_copy(fin[0:STILE, sl3:sl3 + HR],
                                  ps_tiles[3][0:STILE, 0:HR])
            nc.scalar.copy(fin[0:STILE, sl3 + HR:sl3 + ROWS],
                           ps_tiles[3][0:STILE, HR:ROWS])
            # ob on ring A, which is idle once x3A lands
            nc.sync.dma_start(ob_d[:], fin[:, sl3:])

    nc.compile()
    _PROGRAM = nc
    return nc


def _prepare_inputs(timing_dist, singlet_fraction_logit, log_tau_s, log_tau_t,
                    light_oscillation_period, light_response_time, light_gain):
    u = np.ascontiguousarray(np.asarray(timing_dist, dtype=np.float32))
    assert u.shape == (NINPUT, NDET, NTICK)
    gain = np.asarray(light_gain, dtype=np.float32).reshape(NDET)

    c16 = _build_taps(singlet_fraction_logit, log_tau_s, log_tau_t,
                      light_oscillation_period, light_response_time)
    w = _build_weights(c16).reshape(128, NS * WCOL).astype(BF16)

    gain_row = np.tile(gain, N_PER_CORE)                     # [ROWS]

    in_maps = []
    for c in range(NCORES):
        shard = u[c * N_PER_CORE:(c + 1) * N_PER_CORE].reshape(ROWS, NTICK)
        up = np.zeros((ROWS, TALLOC), dtype=np.float32)
        up[:, PAD:TPAD] = shard * gain_row[:, None]
        ub = up.astype(BF16)
        # polyphase relayout: x[q, st, r, row] = ub[row, 16*(100*st+q) + r]
        xv = np.lib.stride_tricks.as_strided(
            ub,
            shape=(128, NST, NS, ROWS),
            strides=(NS * 2, NS * STILE * 2, 2, ub.strides[0]),
        )
        x = np.ascontiguousarray(xv).reshape(128, XFREE)
        in_maps.append({"x": x, "w": w})
    return in_maps


def _run(in_maps, trace=False):
    nc = _build_program()
    res = run_bass_kernel_spmd(nc, in_maps, core_ids=list(range(NCORES)),
                               trace=trace)
    outs = []
    for c in range(NCORES):
        oa = res.results[c]["oa"][0:STILE].astype(np.float32)
        ob = res.results[c]["ob"][0:STILE].astype(np.float32)
        o = np.concatenate(
            [oa.reshape(STILE, 3, ROWS), ob.reshape(STILE, 1, ROWS)],
            axis=1)                                            # [100, 4, 192]
        # out_core[row, s] with s = st*100 + s_rel
        outs.append(np.ascontiguousarray(o.transpose(2, 1, 0))  # [192, 4, 100]
                    .reshape(ROWS, S).reshape(N_PER_CORE, NDET, S))
    full = np.concatenate(outs, axis=0)
    return full, res


def kernel(timing_dist, singlet_fraction_logit, log_tau_s, log_tau_t,
           light_oscillation_period, light_response_time, light_gain):
    in_maps = _prepare_inputs(
        timing_dist, singlet_fraction_logit, log_tau_s, log_tau_t,
        light_oscillation_period, light_response_time, light_gain)
    full, _ = _run(in_maps, trace=False)
    return full
